# revision 7
# baseline (speedup 1.0000x reference)
"""Trainium2 Bass kernel for nn_Dense_BinaryLayer (binary-weight dense layer).

out = x @ Wb + b, where Wb = binarize(W) in {-1, +1}.

Strategy: data-parallel over the 8 NeuronCores — each core handles 2048 rows
of x and the full (replicated) W and b; no collectives.

v2: fp8(e4m3) DoubleRow matmuls.  The PE issues one 512-col matmul every
216ns regardless of dtype (1 output column/cycle), but DoubleRow mode packs
TWO contraction k-tiles per instruction (2 fp8 weights per PE cell), i.e.
2x MAC throughput vs bf16.  Binarized weights are +/-0.5 — EXACT in e4m3 —
so the only quantization error is rounding x to e4m3 (measured 2.5e-2 —
over the 2e-2 gate).  Fix: a RESIDUAL pass over half the contraction:
  - host ships x8 = e4m3(2*x) for all 8 k-tiles (2 MiB/core) and
    r8 = e4m3(2*x - x8) for k-tiles 0-3 (1 MiB/core).  x8+r8 reconstructs
    2x to ~2^-8 relative on those k-tiles; measured end-to-end rel err
    1.86e-2 (numpy sim, deterministic seed) vs gate 2e-2.
  - device contracts 6 DoubleRow k-pair groups per row-tile: 4 hi pairs
    (kt 0-7 of x8) + 2 residual pairs (kt 0-3 of r8), all accumulating in
    one psum group.  The residual pairs REUSE wb k-tiles 0-3 — no extra W
    traffic or binarize.
  - PE busy: 16 row-tiles x 6 groups x 2 (512-col chunks) x 216ns = 41.5us
    vs 55.3us for the bf16 baseline.
Carried over from the bf16 baseline (see git history of this docstring):
  - W ships as bf16 [p, ktile, j] (decision-preserving for the binarize
    threshold 2^-24; fp8 would flip ~500 signs).  binarize per k-tile on
    DVE: wb = (W > 2^-24) - 0.5 in {-0.5, +0.5}, output dtype e4m3; the
    host's exact 2x scaling of x pairs with the +/-0.5 encoding.
  - ALL loads via SWDGE (gpsimd) in strict priority order: W kt0/1,
    x/r pair0, W kt2/3, x/r pair1, W kt4-7, bias, x/r pair2..7; stores
    pre-generate descriptors and park in-queue.
  - ~7 warm-up matmuls during the NEFF preamble walk the PE DVFS ramp
    (1.2GHz -> 2.4GHz after ~3us busy); small fillers reading wb bridge
    supply jitter so the ramp never resets.
  - row-tiles advance in PAIRS (4 psum banks in flight); DVE adds the
    broadcast f32 bias while evicting psum to bf16; last pair runs
    sequentially with a split final store; host upcasts bf16 -> f32.
"""
import sys

sys.path.insert(0, "/opt/trn_rl_repo")

import numpy as np
import ml_dtypes

BF16 = ml_dtypes.bfloat16
F8 = ml_dtypes.float8_e4m3

N_TOTAL = 16384
D_IN = 1024
D_OUT = 1024
N_CORES = 8
ROWS = N_TOTAL // N_CORES      # 2048 rows per core
P = 128
K_TILES = D_IN // P            # 8
R_TILES = K_TILES // 2         # 4 k-tiles carry the residual
I_TILES = ROWS // P            # 16
PAIRS = I_TILES // 2           # 8
BIN_THRESH = 2.0 ** -24
N_WARMUP_MM = 7

_cached = {}


def _build():
    import concourse.tile as tile
    from concourse import bacc, mybir

    f32 = mybir.dt.float32
    bf16 = mybir.dt.bfloat16
    f8 = mybir.dt.float8e4
    TS = mybir.AluOpType
    DR = mybir.MatmulPerfMode.DoubleRow

    nc = bacc.Bacc()
    xt_d = nc.declare_dram_parameter("xT", [I_TILES, P, K_TILES, P], f8,
                                     isOutput=False)
    rt_d = nc.declare_dram_parameter("rT", [I_TILES, P, R_TILES, P], f8,
                                     isOutput=False)
    w_d = nc.declare_dram_parameter("W", [K_TILES, P, D_OUT], bf16,
                                    isOutput=False)
    b_d = nc.declare_dram_parameter("b", [D_OUT], f32, isOutput=False)
    o_d = nc.declare_dram_parameter("out", [I_TILES, P, D_OUT], bf16,
                                    isOutput=True)

    with tile.TileContext(nc) as tc:
        with (
            tc.tile_pool(name="const", bufs=1) as const,
            tc.tile_pool(name="outp", bufs=3) as outp,
            tc.tile_pool(name="pso", bufs=4, space="PSUM") as pso,
        ):
            w_raw = const.tile([P, K_TILES, D_OUT], bf16, tag="wraw")
            xsb = const.tile([P, I_TILES, K_TILES, P], f8, tag="x")
            rsb = const.tile([P, I_TILES, R_TILES, P], f8, tag="r")
            bb = const.tile([P, D_OUT], f32, tag="bb")
            xt_ap = xt_d[:].rearrange("it p kt i -> p it kt i")
            rt_ap = rt_d[:].rearrange("it p kt i -> p it kt i")
            w_ap = w_d[:].rearrange("kt p j -> p kt j")
            nc.gpsimd.dma_start(w_raw[:, 0, :], w_d[0])
            nc.gpsimd.dma_start(w_raw[:, 1, :], w_d[1])
            nc.gpsimd.dma_start(xsb[:, 0:2, :, :], xt_ap[:, 0:2, :, :])
            nc.gpsimd.dma_start(rsb[:, 0:2, :, :], rt_ap[:, 0:2, :, :])
            nc.gpsimd.dma_start(w_raw[:, 2, :], w_d[2])
            nc.gpsimd.dma_start(w_raw[:, 3, :], w_d[3])
            # all of W precedes x/r pair1: pair0 consumes wb k-tiles 4-7
            # ~10us in, long before pair1's x is needed
            nc.gpsimd.dma_start(w_raw[:, 4:6, :], w_ap[:, 4:6, :])
            nc.gpsimd.dma_start(w_raw[:, 6:8, :], w_ap[:, 6:8, :])
            nc.gpsimd.dma_start(xsb[:, 2:4, :, :], xt_ap[:, 2:4, :, :])
            nc.gpsimd.dma_start(rsb[:, 2:4, :, :], rt_ap[:, 2:4, :, :])
            nc.gpsimd.dma_start(bb[:], b_d[:].unsqueeze(0).partition_broadcast(P))
            for pr in range(2, PAIRS):
                nc.gpsimd.dma_start(xsb[:, 2 * pr:2 * pr + 2, :, :],
                                    xt_ap[:, 2 * pr:2 * pr + 2, :, :])
                nc.gpsimd.dma_start(rsb[:, 2 * pr:2 * pr + 2, :, :],
                                    rt_ap[:, 2 * pr:2 * pr + 2, :, :])

            warm = const.tile([P, 512], bf16, tag="warm")
            nc.vector.memset(warm[:], 0.0)

            # binarize: wb[kt] = (W > c) - 0.5 in {-0.5, +0.5} (e4m3 exact).
            wb = const.tile([P, K_TILES, D_OUT], f8, tag="wb")
            for kt in range(K_TILES):
                nc.vector.tensor_scalar(
                    wb[:, kt, :], w_raw[:, kt, :], BIN_THRESH, 0.5,
                    TS.is_gt, TS.subtract,
                )

            warm_ps = pso.tile([P, D_OUT], f32, tag="ps", name="warm_ps")
            for _ in range(N_WARMUP_MM):
                nc.tensor.matmul(warm_ps[:, 0:512], warm[:, 0:P], warm[:],
                                 start=True, stop=True)
            for i in range(9):
                c = (i % 2) * 256
                nc.tensor.matmul(warm_ps[:, 0:256], warm[:, 0:P],
                                 wb[:, 0, c:c + 256], start=True, stop=True)

            def evict(it, ps, cols, suffix=""):
                out_sb = outp.tile([P, D_OUT], bf16, tag="out",
                                   name=f"out_{it}{suffix}")
                for c0, c1 in cols:
                    nc.vector.tensor_tensor(
                        out=out_sb[:, c0:c1], in0=ps[:, c0:c1],
                        in1=bb[:, c0:c1], op=TS.add,
                    )
                    nc.sync.dma_start(o_d[it, :, c0:c1], out_sb[:, c0:c1])

            N_GROUPS = 6  # 4 hi k-pairs + 2 residual k-pairs

            def burst(g, ps_list):
                first = g == 0
                last = g == N_GROUPS - 1
                q = g if g < 4 else g - 4
                src = xsb if g < 4 else rsb
                for it, ps in ps_list:
                    stat = src[:, it, 2 * q:2 * q + 2, :]
                    nc.tensor.matmul(
                        ps[:, 0:512], stat, wb[:, 2 * q:2 * q + 2, 0:512],
                        start=first, stop=last, perf_mode=DR,
                    )
                    nc.tensor.matmul(
                        ps[:, 512:1024], stat,
                        wb[:, 2 * q:2 * q + 2, 512:1024],
                        start=first, stop=last, perf_mode=DR,
                    )

            for pr in range(PAIRS):
                it0, it1 = 2 * pr, 2 * pr + 1
                ps0 = pso.tile([P, D_OUT], f32, tag="ps", name=f"ps_{it0}")
                ps1 = pso.tile([P, D_OUT], f32, tag="ps", name=f"ps_{it1}")
                if pr < PAIRS - 1:
                    for g in range(N_GROUPS):
                        burst(g, ((it0, ps0), (it1, ps1)))
                        if pr == 0 and g in (1, 2):
                            # jitter filler: reads wb (already a dep of the
                            # burst above); keeps the PE busy / DVFS ramp
                            # alive if the next wb k-tile is a hair late
                            nc.tensor.matmul(warm_ps[:, 0:256], warm[:, 0:P],
                                             wb[:, g, 0:256],
                                             start=True, stop=True)
                    # split evictions: halves the DVE latency on the psum
                    # bank reuse edge two pairs later
                    evict(it0, ps0, [(0, 512), (512, D_OUT)])
                    evict(it1, ps1, [(0, 512), (512, D_OUT)])
                else:
                    for g in range(N_GROUPS):
                        burst(g, ((it0, ps0),))
                    evict(it0, ps0, [(0, 512), (512, D_OUT)])
                    # last row-tile column-major: col-half 0's matmuls all
                    # finish first so its eviction+store overlap col-half 1
                    for c0 in (0, 512):
                        for g in range(N_GROUPS):
                            q = g if g < 4 else g - 4
                            src = xsb if g < 4 else rsb
                            nc.tensor.matmul(
                                ps1[:, c0:c0 + 512],
                                src[:, it1, 2 * q:2 * q + 2, :],
                                wb[:, 2 * q:2 * q + 2, c0:c0 + 512],
                                start=(g == 0), stop=(g == N_GROUPS - 1),
                                perf_mode=DR,
                            )
                        evict(it1, ps1, [(c0, c0 + 256),
                                         (c0 + 256, c0 + 512)],
                              suffix=f"_{c0}")

    nc.compile()
    nc.finalize()
    return nc


def _prep_inputs(x, W, b):
    """Host-side shard + layout + dtype split (no arithmetic beyond the exact
    2x scaling and the e4m3 hi/residual decomposition of x)."""
    W16 = np.ascontiguousarray(W.astype(BF16).reshape(K_TILES, P, D_OUT))
    b32 = np.ascontiguousarray(b.astype(np.float32))
    x2 = x * np.float32(2.0)
    x8 = x2.astype(F8)
    r32 = x2 - x8.astype(np.float32)           # exact in f32
    r8 = r32[:, :R_TILES * P].astype(F8)
    in_maps = []
    for c in range(N_CORES):
        sl = slice(c * ROWS, (c + 1) * ROWS)
        t = x8[sl].reshape(I_TILES, P, K_TILES, P).transpose(0, 3, 2, 1)
        rt = r8[sl].reshape(I_TILES, P, R_TILES, P).transpose(0, 3, 2, 1)
        in_maps.append({
            "xT": np.ascontiguousarray(t),
            "rT": np.ascontiguousarray(rt),
            "W": W16,
            "b": b32,
        })
    return in_maps


def kernel(x, W, b):
    from concourse.bass_utils import run_bass_kernel_spmd

    if "nc" not in _cached:
        _cached["nc"] = _build()
    nc = _cached["nc"]

    x = np.asarray(x, dtype=np.float32)
    W = np.asarray(W, dtype=np.float32)
    b = np.asarray(b, dtype=np.float32)

    in_maps = _prep_inputs(x, W, b)
    res = run_bass_kernel_spmd(nc, in_maps, list(range(N_CORES)))
    out = np.concatenate(
        [res.results[c]["out"].reshape(ROWS, D_OUT) for c in range(N_CORES)],
        axis=0,
    )
    return out.astype(np.float32)
